# revision 9
# baseline (speedup 1.0000x reference)
"""MoE batched-experts kernel for Trainium2 (8 NeuronCores, expert-parallel).

Problem: out[n] = sum_e routing[n,e] * MLP_e(x[n]) with N=4096, D=1024,
E=16 experts, H=2048, top-2 routing (routing_tensor is zero except for
each token's 2 chosen experts).

Strategy: only the chosen (token, expert) pairs are computed (8x fewer
FLOPs than the dense reference). Experts are sharded 2-per-core (largest
paired with smallest for load balance). All matmul operands are bf16
(full PE rate, half the HBM traffic of fp32); PSUM accumulation stays
fp32. Every DRAM tensor is laid out [128, big] so each DMA moves multi-KB
contiguous runs per partition (DMA descriptors are per-partition-row;
small rows throttle the queue), and all large DMAs ride the hardware-DGE
queues (sync for inputs/weights, scalar for outputs). A short
dummy-matmul burst at kernel start trips the PE HAM clock gate to
2.4 GHz before data-dependent work begins. Stage 1 computes
hT[h, t] = gelu(x @ W0 + b0) with h on partitions; stage 2 computes
yT[d, t] = W1^T @ hT with d on partitions and tokens on the moving dim
(no 128-token block padding). The routing scale and the scatter-add back
into [N, D] happen on the host.
"""

import math
import os

import numpy as np

P = 128
NE = 16  # experts
D = 1024
H = 2048
NWARM = 6  # dummy matmuls to warm the HAM clock gate at startup

_CACHE: dict = {}
LAST_RESULTS = None  # BassKernelResults of the most recent device run


def _t_chunks(T):
    """Split T into moving-dim chunks <=512; keep every chunk >=256 when
    possible (amortizes per-matmul issue overhead)."""
    out = []
    t = 0
    rem = T
    while rem > 512:
        if rem >= 768:
            out.append((t, 512))
            t += 512
            rem -= 512
        else:  # 513..767: split as (rem-256, 256)
            out.append((t, rem - 256))
            t += rem - 256
            rem = 256
    out.append((t, rem))
    return out


def _build(T0: int, T1: int):
    """Build + compile the 2-expert-per-core MLP program.

    T0/T1: token count for expert slot 0 (large) / 1 (small).
    """
    import concourse.mybir as mybir
    import concourse.tile as tile
    from concourse import bacc

    F32 = mybir.dt.float32
    BF16 = mybir.dt.bfloat16
    AF = mybir.ActivationFunctionType

    KD = D // P   # 8 contraction chunks for x @ W0
    KH = H // P   # 16 contraction chunks for h @ W1
    KC = D // P   # 8 output-row chunks for stage 2 (d on partitions)
    HHALF = H // 2

    nc = bacc.Bacc("TRN2", target_bir_lowering=False, debug=False, num_devices=8)
    # x^T gathered, d-chunk-major: col d*T + t = x[t, d*128:(d+1)*128]
    xt0 = nc.dram_tensor("xt0", [P, KD * T0], BF16, kind="ExternalInput").ap()
    xt1 = nc.dram_tensor("xt1", [P, KD * T1], BF16, kind="ExternalInput").ap()
    # W0: col (half*KD + d)*HHALF + j = W0[d*128+p, half*HHALF + j]
    w0a = nc.dram_tensor("w0a", [P, 2 * KD * HHALF], BF16,
                         kind="ExternalInput").ap()
    w0b = nc.dram_tensor("w0b", [P, 2 * KD * HHALF], BF16,
                         kind="ExternalInput").ap()
    # W1: col h*D + j = W1[h*128+p, j]
    w1a = nc.dram_tensor("w1a", [P, KH * D], BF16, kind="ExternalInput").ap()
    w1b = nc.dram_tensor("w1b", [P, KH * D], BF16, kind="ExternalInput").ap()
    # b0 per slot: [p, 2*KH] (col s*KH+h = b0[h*128:(h+1)*128] of slot s)
    b0s = nc.dram_tensor("b0s", [P, 2 * KH], F32, kind="ExternalInput").ap()
    # yT per slot, dc-major: col dc*T + t = y[t, dc*128+p] (host applies the
    # routing score)
    yt0 = nc.dram_tensor("yt0", [P, KC * T0], F32, kind="ExternalOutput").ap()
    yt1 = nc.dram_tensor("yt1", [P, KC * T1], F32, kind="ExternalOutput").ap()

    with tile.TileContext(nc) as tc:
        with tc.tile_pool(name="wp", bufs=1) as wp, \
             tc.tile_pool(name="dp", bufs=1) as dp, \
             tc.tile_pool(name="op", bufs=1) as op, \
             tc.tile_pool(name="ps", bufs=8, space="PSUM") as ps:
            # --- HAM warm-up: keep the PE busy from program start so the
            # clock gate reaches 8/8 (2.4 GHz) before data-dependent
            # matmuls begin (cold matmuls run at 1.2 GHz).
            wd = dp.tile([P, 512], BF16, tag="wu")
            nc.gpsimd.memset(wd[:], 0.0)
            psd = ps.tile([P, 512], F32, tag="ps", name="ps_warm")
            for i in range(NWARM):
                nc.tensor.matmul(psd[:], wd[:, :P], wd[:], start=True, stop=True)

            # --- startup DMAs (small slot runs first: its 512-wide chunks
            # consume weights at a rate the DMA stream can sustain).
            # Fine-grained first pieces so the d-accumulation starts as soon
            # as one (x, w0) pair lands, then just-in-time per-d pieces.
            xb = dp.tile([P, KD * T1], BF16, tag="xb")
            w0sb = wp.tile([P, 2 * KD * HHALF], BF16, tag="w0")
            w1sb = wp.tile([P, KH * D], BF16, tag="w1")
            xa = dp.tile([P, KD * T0], BF16, tag="xa")
            nc.sync.dma_start(w0sb[:, :HHALF], w0b[:, :HHALF])
            nc.sync.dma_start(xb[:, :2 * T1], xt1[:, :2 * T1])
            nc.sync.dma_start(w0sb[:, HHALF:2 * HHALF], w0b[:, HHALF:2 * HHALF])
            nc.sync.dma_start(w0sb[:, 2 * HHALF:4 * HHALF],
                              w0b[:, 2 * HHALF:4 * HHALF])
            nc.sync.dma_start(xb[:, 2 * T1:], xt1[:, 2 * T1:])
            nc.sync.dma_start(w0sb[:, 4 * HHALF:6 * HHALF],
                              w0b[:, 4 * HHALF:6 * HHALF])
            nc.sync.dma_start(w0sb[:, 6 * HHALF:KD * HHALF],
                              w0b[:, 6 * HHALF:KD * HHALF])
            nc.sync.dma_start(w0sb[:, KD * HHALF:], w0b[:, KD * HHALF:])
            nc.sync.dma_start(w1sb[:, :KH * D // 2], w1b[:, :KH * D // 2])
            nc.sync.dma_start(w1sb[:, KH * D // 2:], w1b[:, KH * D // 2:])
            nc.sync.dma_start(xa[:, :4 * T0], xt0[:, :4 * T0])
            nc.sync.dma_start(xa[:, 4 * T0:], xt0[:, 4 * T0:])
            b0t = dp.tile([P, 2 * KH], F32, tag="b0t")
            nc.gpsimd.dma_start(b0t[:], b0s)

            for s, (w0_in, w1_in, yt_out, xsb, T) in enumerate(
                [(w0b, w1b, yt1, xb, T1), (w0a, w1a, yt0, xa, T0)]
            ):
                if s == 1:
                    # big-slot weights reuse the small slot's SBUF (WAR deps
                    # pipeline the transfers behind the last reader)
                    nc.sync.dma_start(w0sb[:, :KD * HHALF],
                                      w0_in[:, :KD * HHALF])
                    nc.sync.dma_start(w0sb[:, KD * HHALF:],
                                      w0_in[:, KD * HHALF:])
                    nc.sync.dma_start(w1sb[:, :KH * D // 2],
                                      w1_in[:, :KH * D // 2])
                    nc.sync.dma_start(w1sb[:, KH * D // 2:],
                                      w1_in[:, KH * D // 2:])
                hts = []
                for h in range(KH):
                    ht = dp.tile([P, T], BF16, tag=f"ht{h}", name=f"ht_s{s}_{h}")
                    hts.append(ht)

                # --- stage 1: hT[h, t] = gelu(x @ W0 + b0), h on partitions.
                # half-outer so W0-half0 is fully consumed (both t-chunks)
                # before half1 is needed -> streams at sustainable DMA rate.
                for half in range(2):
                    for (t0, tw) in _t_chunks(T):
                        pt = []
                        for hh in range(KH // 2):
                            p_ = ps.tile([P, 512], F32, tag="ps",
                                         name=f"ps1_s{s}_{half}_{t0}_{hh}")
                            pt.append(p_)
                        for d in range(KD):
                            w0off = (half * KD + d) * HHALF
                            for hh in range(KH // 2):
                                nc.tensor.matmul(
                                    pt[hh][:, :tw],
                                    w0sb[:, w0off + hh * P: w0off + (hh + 1) * P],
                                    xsb[:, d * T + t0: d * T + t0 + tw],
                                    start=(d == 0), stop=(d == KD - 1),
                                )
                        for hh in range(KH // 2):
                            h = half * (KH // 2) + hh
                            nc.scalar.activation(
                                hts[h][:, t0:t0 + tw], pt[hh][:, :tw],
                                AF.Gelu, bias=b0t[:, s * KH + h: s * KH + h + 1],
                            )

                # --- stage 2: yT[d, t] = W1^T @ hT, d on partitions, tokens
                # moving (no 128-token block padding). dc-outer so the
                # output can stream out in two contiguous half-D DMAs.
                yb = op.tile([P, KC * T], F32, tag="yb", name=f"yb_s{s}")
                for dc in range(KC):
                    for (t0, tw) in _t_chunks(T):
                        p2 = ps.tile([P, 512], F32, tag="ps",
                                     name=f"ps2_s{s}_{dc}_{t0}")
                        for h in range(KH):
                            nc.tensor.matmul(
                                p2[:, :tw],
                                w1sb[:, h * D + dc * P: h * D + (dc + 1) * P],
                                hts[h][:, t0:t0 + tw],
                                start=(h == 0), stop=(h == KH - 1),
                            )
                        nc.vector.tensor_copy(
                            yb[:, dc * T + t0: dc * T + t0 + tw], p2[:, :tw])
                    if dc % 2 == 1:
                        nc.scalar.dma_start(
                            yt_out[:, (dc - 1) * T:(dc + 1) * T],
                            yb[:, (dc - 1) * T:(dc + 1) * T])

    nc.compile()
    return nc


def _ensure_ntff_hook_module():
    """bass_utils unconditionally imports antenv.axon_hooks when tracing is
    requested; on images without it, provide a shim wired to the axon
    ctypes profiler when available (else a no-hook fallback)."""
    import importlib.util
    import sys
    import types

    if importlib.util.find_spec("antenv") is None:
        return
    try:
        import antenv.axon_hooks  # noqa: F401
        return
    except ImportError:
        pass
    mod = types.ModuleType("antenv.axon_hooks")
    mod._hook = None

    def set_axon_ntff_profile_hook(h):
        mod._hook = h

    def get_axon_ntff_profile_hook():
        return mod._hook

    mod.set_axon_ntff_profile_hook = set_axon_ntff_profile_hook
    mod.get_axon_ntff_profile_hook = get_axon_ntff_profile_hook
    try:
        from trn_agent_boot.trn_boot import _ntff_profile_via_ctypes
        mod._hook = _ntff_profile_via_ctypes("/opt/axon/libaxon_pjrt.so")
    except Exception:
        pass
    sys.modules["antenv.axon_hooks"] = mod
    import antenv
    antenv.axon_hooks = mod


def kernel(x, routing_tensor, W0, b0, W1, b1):
    global LAST_RESULTS
    import ml_dtypes
    from concourse.bass_utils import run_bass_kernel_spmd
    _ensure_ntff_hook_module()

    BF = ml_dtypes.bfloat16
    x = np.ascontiguousarray(x, dtype=np.float32)
    routing = np.asarray(routing_tensor, dtype=np.float32)
    W0 = np.asarray(W0, dtype=np.float32)
    b0 = np.asarray(b0, dtype=np.float32)
    W1 = np.asarray(W1, dtype=np.float32)
    b1 = np.asarray(b1, dtype=np.float32)

    # --- routing prep: per-expert token lists ---
    idx = [np.nonzero(routing[:, e])[0] for e in range(NE)]
    counts = np.array([len(i) for i in idx])
    order = np.argsort(-counts, kind="stable")  # experts sorted by load desc
    big, small = order[:8], order[8:][::-1]     # pair rank i with rank 15-i
    T0 = max(P, int(counts[big].max()) + 1 >> 1 << 1)
    T1 = max(P, int(counts[small].max()) + 1 >> 1 << 1)

    key = (T0, T1)
    if key not in _CACHE:
        _CACHE[key] = _build(T0, T1)
    nc = _CACHE[key]

    # --- build per-core inputs ---
    in_maps = []
    KD, KH, HHALF = D // P, H // P, H // 2
    for c in range(8):
        ea, eb = int(big[c]), int(small[c])
        m = {}
        for name, e, T in (("xt0", ea, T0), ("xt1", eb, T1)):
            g = np.zeros((T, D), np.float32)
            g[: len(idx[e])] = x[idx[e]]
            # [P, KD*T]: col d*T + t = x[t, d*128+p]
            m[name] = np.ascontiguousarray(
                g.T.reshape(KD, P, T).transpose(1, 0, 2).reshape(P, KD * T)
                .astype(BF))
        for name, e in (("w0a", ea), ("w0b", eb)):
            # [P, 2*KD*HHALF]: col (half*KD+d)*HHALF + j
            m[name] = np.ascontiguousarray(
                W0[e].reshape(KD, P, 2, HHALF).transpose(1, 2, 0, 3)
                .reshape(P, 2 * KD * HHALF).astype(BF))
        for name, e in (("w1a", ea), ("w1b", eb)):
            # [P, KH*D]: col h*D + j
            m[name] = np.ascontiguousarray(
                W1[e].reshape(KH, P, D).transpose(1, 0, 2)
                .reshape(P, KH * D).astype(BF))
        # device processes the small slot (eb) first: its bias in cols [:KH]
        b0m = np.zeros((P, 2 * KH), np.float32)
        b0m[:, :KH] = b0[eb].reshape(KH, P).T
        b0m[:, KH:] = b0[ea].reshape(KH, P).T
        m["b0s"] = b0m
        in_maps.append(m)

    res = run_bass_kernel_spmd(nc, in_maps, core_ids=list(range(8)),
                               trace=bool(os.environ.get("BASS_TRACE")))
    LAST_RESULTS = res

    # --- combine: out = routing @ b1 + score-scaled scatter-add of the
    # per-expert transposed outputs ---
    out = routing @ b1
    for c in range(8):
        ea, eb = int(big[c]), int(small[c])
        # yt [P, KC*T] -> y^T [D, T]: y^T[dc*128+p, t] = yt[p, dc*T+t]
        y0 = np.asarray(res.results[c]["yt0"]).reshape(P, D // P, T0) \
            .transpose(1, 0, 2).reshape(D, T0)
        y1 = np.asarray(res.results[c]["yt1"]).reshape(P, D // P, T1) \
            .transpose(1, 0, 2).reshape(D, T1)
        na, nb = len(idx[ea]), len(idx[eb])
        out[idx[ea]] += y0[:, :na].T * routing[idx[ea], ea][:, None]
        out[idx[eb]] += y1[:, :nb].T * routing[idx[eb], eb][:, None]
    return out.astype(np.float32)


# revision 10
# speedup vs baseline: 1.1894x; 1.1894x over previous
"""MoE batched-experts kernel for Trainium2 (8 NeuronCores, expert-parallel).

Problem: out[n] = sum_e routing[n,e] * MLP_e(x[n]) with N=4096, D=1024,
E=16 experts, H=2048, top-2 routing (routing_tensor is zero except for
each token's 2 chosen experts).

Strategy: only the chosen (token, expert) pairs are computed (8x fewer
FLOPs than the dense reference). Experts are sharded 2-per-core (largest
paired with smallest for load balance). All matmul operands are bf16
(full PE rate, half the HBM traffic of fp32); PSUM accumulation stays
fp32. Every DRAM tensor is laid out [128, big] so each DMA moves multi-KB
contiguous runs per partition (DMA descriptors are per-partition-row;
small rows throttle the queue), and all large DMAs ride the hardware-DGE
queues (sync for inputs/weights, scalar for outputs). A short
dummy-matmul burst at kernel start trips the PE HAM clock gate to
2.4 GHz before data-dependent work begins. Stage 1 computes
hT[h, t] = gelu(x @ W0 + b0) with h on partitions; stage 2 computes
yT[d, t] = W1^T @ hT with d on partitions and tokens on the moving dim
(no 128-token block padding). The routing scale and the scatter-add back
into [N, D] happen on the host.
"""

import math
import os

import numpy as np

P = 128
NE = 16  # experts
D = 1024
H = 2048
NWARM = 6  # dummy matmuls to warm the HAM clock gate at startup

_CACHE: dict = {}
LAST_RESULTS = None  # BassKernelResults of the most recent device run


def _t_chunks(T):
    """Split T into moving-dim chunks <=512; keep every chunk >=256 when
    possible (amortizes per-matmul issue overhead)."""
    out = []
    t = 0
    rem = T
    while rem > 512:
        if rem >= 768:
            out.append((t, 512))
            t += 512
            rem -= 512
        else:  # 513..767: split as (rem-256, 256)
            out.append((t, rem - 256))
            t += rem - 256
            rem = 256
    out.append((t, rem))
    return out


def _build(T0: int, T1: int):
    """Build + compile the 2-expert-per-core MLP program.

    T0/T1: token count for expert slot 0 (large) / 1 (small).
    """
    import concourse.mybir as mybir
    import concourse.tile as tile
    from concourse import bacc

    F32 = mybir.dt.float32
    BF16 = mybir.dt.bfloat16
    AF = mybir.ActivationFunctionType

    KD = D // P   # 8 contraction chunks for x @ W0
    KH = H // P   # 16 contraction chunks for h @ W1
    KC = D // P   # 8 output-row chunks for stage 2 (d on partitions)
    HHALF = H // 2

    nc = bacc.Bacc("TRN2", target_bir_lowering=False, debug=False, num_devices=8)
    # x^T gathered, d-chunk-major: col d*T + t = x[t, d*128:(d+1)*128]
    xt0 = nc.dram_tensor("xt0", [P, KD * T0], BF16, kind="ExternalInput").ap()
    xt1 = nc.dram_tensor("xt1", [P, KD * T1], BF16, kind="ExternalInput").ap()
    # W0: col (half*KD + d)*HHALF + j = W0[d*128+p, half*HHALF + j]
    w0a = nc.dram_tensor("w0a", [P, 2 * KD * HHALF], BF16,
                         kind="ExternalInput").ap()
    w0b = nc.dram_tensor("w0b", [P, 2 * KD * HHALF], BF16,
                         kind="ExternalInput").ap()
    # W1: col h*D + j = W1[h*128+p, j]
    w1a = nc.dram_tensor("w1a", [P, KH * D], BF16, kind="ExternalInput").ap()
    w1b = nc.dram_tensor("w1b", [P, KH * D], BF16, kind="ExternalInput").ap()
    # b0 per slot: [p, 2*KH] (col s*KH+h = b0[h*128:(h+1)*128] of slot s)
    b0s = nc.dram_tensor("b0s", [P, 2 * KH], F32, kind="ExternalInput").ap()
    # yT per slot, dc-major: col dc*T + t = y[t, dc*128+p] (host applies the
    # routing score)
    yt0 = nc.dram_tensor("yt0", [P, KC * T0], F32, kind="ExternalOutput").ap()
    yt1 = nc.dram_tensor("yt1", [P, KC * T1], F32, kind="ExternalOutput").ap()

    with tile.TileContext(nc) as tc:
        with tc.tile_pool(name="wp", bufs=1) as wp, \
             tc.tile_pool(name="dp", bufs=1) as dp, \
             tc.tile_pool(name="op", bufs=1) as op, \
             tc.tile_pool(name="ps", bufs=8, space="PSUM") as ps:
            # --- HAM warm-up: keep the PE busy from program start so the
            # clock gate reaches 8/8 (2.4 GHz) before data-dependent
            # matmuls begin (cold matmuls run at 1.2 GHz).
            wd = dp.tile([P, 512], BF16, tag="wu")
            nc.gpsimd.memset(wd[:], 0.0)
            psd = ps.tile([P, 512], F32, tag="ps", name="ps_warm")
            for i in range(NWARM):
                nc.tensor.matmul(psd[:], wd[:, :P], wd[:], start=True, stop=True)

            # --- startup DMAs (small slot runs first: its 512-wide chunks
            # consume weights at a rate the DMA stream can sustain).
            # Fine-grained first pieces so the d-accumulation starts as soon
            # as one (x, w0) pair lands, then just-in-time per-d pieces.
            xb = dp.tile([P, KD * T1], BF16, tag="xb")
            w0sb = wp.tile([P, 2 * KD * HHALF], BF16, tag="w0")
            w1sb = wp.tile([P, KH * D], BF16, tag="w1")
            xa = dp.tile([P, KD * T0], BF16, tag="xa")
            nc.sync.dma_start(w0sb[:, :HHALF], w0b[:, :HHALF])
            nc.sync.dma_start(xb[:, :T1], xt1[:, :T1])
            nc.sync.dma_start(w0sb[:, HHALF:2 * HHALF], w0b[:, HHALF:2 * HHALF])
            nc.sync.dma_start(xb[:, T1:2 * T1], xt1[:, T1:2 * T1])
            nc.sync.dma_start(w0sb[:, 2 * HHALF:4 * HHALF],
                              w0b[:, 2 * HHALF:4 * HHALF])
            nc.sync.dma_start(xb[:, 2 * T1:4 * T1], xt1[:, 2 * T1:4 * T1])
            nc.sync.dma_start(w0sb[:, 4 * HHALF:6 * HHALF],
                              w0b[:, 4 * HHALF:6 * HHALF])
            nc.sync.dma_start(xb[:, 4 * T1:], xt1[:, 4 * T1:])
            nc.sync.dma_start(w0sb[:, 6 * HHALF:KD * HHALF],
                              w0b[:, 6 * HHALF:KD * HHALF])
            nc.sync.dma_start(w0sb[:, KD * HHALF:], w0b[:, KD * HHALF:])
            nc.sync.dma_start(w1sb[:, :KH * D // 2], w1b[:, :KH * D // 2])
            nc.sync.dma_start(w1sb[:, KH * D // 2:], w1b[:, KH * D // 2:])
            nc.sync.dma_start(xa[:, :4 * T0], xt0[:, :4 * T0])
            nc.sync.dma_start(xa[:, 4 * T0:], xt0[:, 4 * T0:])
            b0t = dp.tile([P, 2 * KH], F32, tag="b0t")
            nc.gpsimd.dma_start(b0t[:], b0s)

            for s, (w0_in, w1_in, yt_out, xsb, T) in enumerate(
                [(w0b, w1b, yt1, xb, T1), (w0a, w1a, yt0, xa, T0)]
            ):
                if s == 1:
                    # big-slot weights reuse the small slot's SBUF (WAR deps
                    # pipeline the transfers behind the last reader)
                    nc.sync.dma_start(w0sb[:, :KD * HHALF],
                                      w0_in[:, :KD * HHALF])
                    nc.sync.dma_start(w0sb[:, KD * HHALF:],
                                      w0_in[:, KD * HHALF:])
                    nc.sync.dma_start(w1sb[:, :KH * D // 2],
                                      w1_in[:, :KH * D // 2])
                    nc.sync.dma_start(w1sb[:, KH * D // 2:],
                                      w1_in[:, KH * D // 2:])
                hts = []
                for h in range(KH):
                    ht = dp.tile([P, T], BF16, tag=f"ht{h}", name=f"ht_s{s}_{h}")
                    hts.append(ht)

                # --- stage 1: hT[h, t] = gelu(x @ W0 + b0), h on partitions.
                # half-outer so W0-half0 is fully consumed (both t-chunks)
                # before half1 is needed -> streams at sustainable DMA rate.
                for half in range(2):
                    for (t0, tw) in _t_chunks(T):
                        pt = []
                        for hh in range(KH // 2):
                            p_ = ps.tile([P, 512], F32, tag="ps",
                                         name=f"ps1_s{s}_{half}_{t0}_{hh}")
                            pt.append(p_)
                        for d in range(KD):
                            w0off = (half * KD + d) * HHALF
                            for hh in range(KH // 2):
                                nc.tensor.matmul(
                                    pt[hh][:, :tw],
                                    w0sb[:, w0off + hh * P: w0off + (hh + 1) * P],
                                    xsb[:, d * T + t0: d * T + t0 + tw],
                                    start=(d == 0), stop=(d == KD - 1),
                                )
                        for hh in range(KH // 2):
                            h = half * (KH // 2) + hh
                            nc.scalar.activation(
                                hts[h][:, t0:t0 + tw], pt[hh][:, :tw],
                                AF.Gelu, bias=b0t[:, s * KH + h: s * KH + h + 1],
                            )

                # --- stage 2: yT[d, t] = W1^T @ hT, d on partitions, tokens
                # moving (no 128-token block padding). dc-outer so the
                # output can stream out in two contiguous half-D DMAs.
                yb = op.tile([P, KC * T], F32, tag="yb", name=f"yb_s{s}")
                for dc in range(KC):
                    for (t0, tw) in _t_chunks(T):
                        p2 = ps.tile([P, 512], F32, tag="ps",
                                     name=f"ps2_s{s}_{dc}_{t0}")
                        for h in range(KH):
                            nc.tensor.matmul(
                                p2[:, :tw],
                                w1sb[:, h * D + dc * P: h * D + (dc + 1) * P],
                                hts[h][:, t0:t0 + tw],
                                start=(h == 0), stop=(h == KH - 1),
                            )
                        nc.vector.tensor_copy(
                            yb[:, dc * T + t0: dc * T + t0 + tw], p2[:, :tw])
                    if dc % 2 == 1:
                        nc.scalar.dma_start(
                            yt_out[:, (dc - 1) * T:(dc + 1) * T],
                            yb[:, (dc - 1) * T:(dc + 1) * T])

    nc.compile()
    return nc


def _ensure_ntff_hook_module():
    """bass_utils unconditionally imports antenv.axon_hooks when tracing is
    requested; on images without it, provide a shim wired to the axon
    ctypes profiler when available (else a no-hook fallback)."""
    import importlib.util
    import sys
    import types

    if importlib.util.find_spec("antenv") is None:
        return
    try:
        import antenv.axon_hooks  # noqa: F401
        return
    except ImportError:
        pass
    mod = types.ModuleType("antenv.axon_hooks")
    mod._hook = None

    def set_axon_ntff_profile_hook(h):
        mod._hook = h

    def get_axon_ntff_profile_hook():
        return mod._hook

    mod.set_axon_ntff_profile_hook = set_axon_ntff_profile_hook
    mod.get_axon_ntff_profile_hook = get_axon_ntff_profile_hook
    try:
        from trn_agent_boot.trn_boot import _ntff_profile_via_ctypes
        mod._hook = _ntff_profile_via_ctypes("/opt/axon/libaxon_pjrt.so")
    except Exception:
        pass
    sys.modules["antenv.axon_hooks"] = mod
    import antenv
    antenv.axon_hooks = mod


def kernel(x, routing_tensor, W0, b0, W1, b1):
    global LAST_RESULTS
    import ml_dtypes
    from concourse.bass_utils import run_bass_kernel_spmd
    _ensure_ntff_hook_module()

    BF = ml_dtypes.bfloat16
    x = np.ascontiguousarray(x, dtype=np.float32)
    routing = np.asarray(routing_tensor, dtype=np.float32)
    W0 = np.asarray(W0, dtype=np.float32)
    b0 = np.asarray(b0, dtype=np.float32)
    W1 = np.asarray(W1, dtype=np.float32)
    b1 = np.asarray(b1, dtype=np.float32)

    # --- routing prep: per-expert token lists ---
    idx = [np.nonzero(routing[:, e])[0] for e in range(NE)]
    counts = np.array([len(i) for i in idx])
    order = np.argsort(-counts, kind="stable")  # experts sorted by load desc
    big, small = order[:8], order[8:][::-1]     # pair rank i with rank 15-i
    T0 = max(P, int(counts[big].max()) + 1 >> 1 << 1)
    T1 = max(P, int(counts[small].max()) + 1 >> 1 << 1)

    key = (T0, T1)
    if key not in _CACHE:
        _CACHE[key] = _build(T0, T1)
    nc = _CACHE[key]

    # --- build per-core inputs ---
    in_maps = []
    KD, KH, HHALF = D // P, H // P, H // 2
    for c in range(8):
        ea, eb = int(big[c]), int(small[c])
        m = {}
        for name, e, T in (("xt0", ea, T0), ("xt1", eb, T1)):
            g = np.zeros((T, D), np.float32)
            g[: len(idx[e])] = x[idx[e]]
            # [P, KD*T]: col d*T + t = x[t, d*128+p]
            m[name] = np.ascontiguousarray(
                g.T.reshape(KD, P, T).transpose(1, 0, 2).reshape(P, KD * T)
                .astype(BF))
        for name, e in (("w0a", ea), ("w0b", eb)):
            # [P, 2*KD*HHALF]: col (half*KD+d)*HHALF + j
            m[name] = np.ascontiguousarray(
                W0[e].reshape(KD, P, 2, HHALF).transpose(1, 2, 0, 3)
                .reshape(P, 2 * KD * HHALF).astype(BF))
        for name, e in (("w1a", ea), ("w1b", eb)):
            # [P, KH*D]: col h*D + j
            m[name] = np.ascontiguousarray(
                W1[e].reshape(KH, P, D).transpose(1, 0, 2)
                .reshape(P, KH * D).astype(BF))
        # device processes the small slot (eb) first: its bias in cols [:KH]
        b0m = np.zeros((P, 2 * KH), np.float32)
        b0m[:, :KH] = b0[eb].reshape(KH, P).T
        b0m[:, KH:] = b0[ea].reshape(KH, P).T
        m["b0s"] = b0m
        in_maps.append(m)

    res = run_bass_kernel_spmd(nc, in_maps, core_ids=list(range(8)),
                               trace=bool(os.environ.get("BASS_TRACE")))
    LAST_RESULTS = res

    # --- combine: out = routing @ b1 + score-scaled scatter-add of the
    # per-expert transposed outputs ---
    out = routing @ b1
    for c in range(8):
        ea, eb = int(big[c]), int(small[c])
        # yt [P, KC*T] -> y^T [D, T]: y^T[dc*128+p, t] = yt[p, dc*T+t]
        y0 = np.asarray(res.results[c]["yt0"]).reshape(P, D // P, T0) \
            .transpose(1, 0, 2).reshape(D, T0)
        y1 = np.asarray(res.results[c]["yt1"]).reshape(P, D // P, T1) \
            .transpose(1, 0, 2).reshape(D, T1)
        na, nb = len(idx[ea]), len(idx[eb])
        out[idx[ea]] += y0[:, :na].T * routing[idx[ea], ea][:, None]
        out[idx[eb]] += y1[:, :nb].T * routing[idx[eb], eb][:, None]
    return out.astype(np.float32)


# revision 12
# speedup vs baseline: 1.1954x; 1.0050x over previous
"""MoE batched-experts kernel for Trainium2 (8 NeuronCores, expert-parallel).

Problem: out[n] = sum_e routing[n,e] * MLP_e(x[n]) with N=4096, D=1024,
E=16 experts, H=2048, top-2 routing (routing_tensor is zero except for
each token's 2 chosen experts).

Strategy: only the chosen (token, expert) pairs are computed (8x fewer
FLOPs than the dense reference). Experts are sharded 2-per-core (largest
paired with smallest for load balance). All matmul operands are bf16
(full PE rate, half the HBM traffic of fp32); PSUM accumulation stays
fp32. Every DRAM tensor is laid out [128, big] so each DMA moves multi-KB
contiguous runs per partition (DMA descriptors are per-partition-row;
small rows throttle the queue), and all large DMAs ride the hardware-DGE
queues (sync for inputs/weights, scalar for outputs). A short
dummy-matmul burst at kernel start trips the PE HAM clock gate to
2.4 GHz before data-dependent work begins. Stage 1 computes
hT[h, t] = gelu(x @ W0 + b0) with h on partitions; stage 2 computes
yT[d, t] = W1^T @ hT with d on partitions and tokens on the moving dim
(no 128-token block padding). The routing scale and the scatter-add back
into [N, D] happen on the host.
"""

import math
import os

import numpy as np

P = 128
NE = 16  # experts
D = 1024
H = 2048
NWARM = 5  # dummy matmuls to warm the HAM clock gate at startup

_CACHE: dict = {}
LAST_RESULTS = None  # BassKernelResults of the most recent device run


def _t_chunks(T):
    """Split T into moving-dim chunks <=512; keep every chunk >=256 when
    possible (amortizes per-matmul issue overhead)."""
    out = []
    t = 0
    rem = T
    while rem > 512:
        if rem >= 768:
            out.append((t, 512))
            t += 512
            rem -= 512
        else:  # 513..767: split as (rem-256, 256)
            out.append((t, rem - 256))
            t += rem - 256
            rem = 256
    out.append((t, rem))
    return out


def _build(T0: int, T1: int):
    """Build + compile the 2-expert-per-core MLP program.

    T0/T1: token count for expert slot 0 (large) / 1 (small).
    """
    import concourse.mybir as mybir
    import concourse.tile as tile
    from concourse import bacc

    F32 = mybir.dt.float32
    BF16 = mybir.dt.bfloat16
    AF = mybir.ActivationFunctionType

    KD = D // P   # 8 contraction chunks for x @ W0
    KH = H // P   # 16 contraction chunks for h @ W1
    KC = D // P   # 8 output-row chunks for stage 2 (d on partitions)
    HHALF = H // 2

    nc = bacc.Bacc("TRN2", target_bir_lowering=False, debug=False, num_devices=8)
    # x^T gathered, d-chunk-major: col d*T + t = x[t, d*128:(d+1)*128]
    xt0 = nc.dram_tensor("xt0", [P, KD * T0], BF16, kind="ExternalInput").ap()
    xt1 = nc.dram_tensor("xt1", [P, KD * T1], BF16, kind="ExternalInput").ap()
    # W0: col (half*KD + d)*HHALF + j = W0[d*128+p, half*HHALF + j]
    w0a = nc.dram_tensor("w0a", [P, 2 * KD * HHALF], BF16,
                         kind="ExternalInput").ap()
    w0b = nc.dram_tensor("w0b", [P, 2 * KD * HHALF], BF16,
                         kind="ExternalInput").ap()
    # W1: col h*D + j = W1[h*128+p, j]
    w1a = nc.dram_tensor("w1a", [P, KH * D], BF16, kind="ExternalInput").ap()
    w1b = nc.dram_tensor("w1b", [P, KH * D], BF16, kind="ExternalInput").ap()
    # b0 per slot: [p, 2*KH] (col s*KH+h = b0[h*128:(h+1)*128] of slot s)
    b0s = nc.dram_tensor("b0s", [P, 2 * KH], F32, kind="ExternalInput").ap()
    # yT per slot, dc-major: col dc*T + t = y[t, dc*128+p] (host applies the
    # routing score)
    yt0 = nc.dram_tensor("yt0", [P, KC * T0], F32, kind="ExternalOutput").ap()
    yt1 = nc.dram_tensor("yt1", [P, KC * T1], F32, kind="ExternalOutput").ap()

    with tile.TileContext(nc) as tc:
        with tc.tile_pool(name="wp", bufs=1) as wp, \
             tc.tile_pool(name="dp", bufs=1) as dp, \
             tc.tile_pool(name="op", bufs=1) as op, \
             tc.tile_pool(name="ps", bufs=8, space="PSUM") as ps:
            # --- HAM warm-up: keep the PE busy from program start so the
            # clock gate reaches 8/8 (2.4 GHz) before data-dependent
            # matmuls begin (cold matmuls run at 1.2 GHz).
            wd = dp.tile([P, 512], BF16, tag="wu")
            nc.gpsimd.memset(wd[:], 0.0)
            psd = ps.tile([P, 512], F32, tag="ps", name="ps_warm")
            for i in range(NWARM):
                nc.tensor.matmul(psd[:], wd[:, :P], wd[:], start=True, stop=True)

            # --- startup DMAs (small slot runs first: its 512-wide chunks
            # consume weights at a rate the DMA stream can sustain).
            # Fine-grained first pieces so the d-accumulation starts as soon
            # as one (x, w0) pair lands, then just-in-time per-d pieces.
            xb = dp.tile([P, KD * T1], BF16, tag="xb")
            w0sb = wp.tile([P, 2 * KD * HHALF], BF16, tag="w0")
            w1sb = wp.tile([P, KH * D], BF16, tag="w1")
            xa = dp.tile([P, KD * T0], BF16, tag="xa")
            nc.sync.dma_start(w0sb[:, :HHALF], w0b[:, :HHALF])
            nc.sync.dma_start(xb[:, :T1], xt1[:, :T1])
            nc.sync.dma_start(w0sb[:, HHALF:2 * HHALF], w0b[:, HHALF:2 * HHALF])
            nc.sync.dma_start(xb[:, T1:2 * T1], xt1[:, T1:2 * T1])
            nc.sync.dma_start(w0sb[:, 2 * HHALF:4 * HHALF],
                              w0b[:, 2 * HHALF:4 * HHALF])
            nc.sync.dma_start(xb[:, 2 * T1:4 * T1], xt1[:, 2 * T1:4 * T1])
            nc.sync.dma_start(w0sb[:, 4 * HHALF:6 * HHALF],
                              w0b[:, 4 * HHALF:6 * HHALF])
            nc.sync.dma_start(xb[:, 4 * T1:], xt1[:, 4 * T1:])
            nc.sync.dma_start(w0sb[:, 6 * HHALF:KD * HHALF],
                              w0b[:, 6 * HHALF:KD * HHALF])
            nc.sync.dma_start(w0sb[:, KD * HHALF:], w0b[:, KD * HHALF:])
            nc.sync.dma_start(w1sb[:, :KH * D // 2], w1b[:, :KH * D // 2])
            nc.sync.dma_start(w1sb[:, KH * D // 2:], w1b[:, KH * D // 2:])
            nc.sync.dma_start(xa[:, :4 * T0], xt0[:, :4 * T0])
            nc.sync.dma_start(xa[:, 4 * T0:], xt0[:, 4 * T0:])
            b0t = dp.tile([P, 2 * KH], F32, tag="b0t")
            nc.gpsimd.dma_start(b0t[:], b0s)

            for s, (w0_in, w1_in, yt_out, xsb, T) in enumerate(
                [(w0b, w1b, yt1, xb, T1), (w0a, w1a, yt0, xa, T0)]
            ):
                if s == 1:
                    # big-slot weights reuse the small slot's SBUF (WAR deps
                    # pipeline the transfers behind the last reader)
                    nc.sync.dma_start(w0sb[:, :KD * HHALF],
                                      w0_in[:, :KD * HHALF])
                    nc.sync.dma_start(w0sb[:, KD * HHALF:],
                                      w0_in[:, KD * HHALF:])
                    nc.sync.dma_start(w1sb[:, :KH * D // 2],
                                      w1_in[:, :KH * D // 2])
                    nc.sync.dma_start(w1sb[:, KH * D // 2:],
                                      w1_in[:, KH * D // 2:])
                hts = []
                for h in range(KH):
                    ht = dp.tile([P, T], BF16, tag=f"ht{h}", name=f"ht_s{s}_{h}")
                    hts.append(ht)

                # --- stage 1: hT[h, t] = gelu(x @ W0 + b0), h on partitions.
                # half-outer so W0-half0 is fully consumed (both t-chunks)
                # before half1 is needed -> streams at sustainable DMA rate.
                for half in range(2):
                    for (t0, tw) in _t_chunks(T):
                        pt = []
                        for hh in range(KH // 2):
                            p_ = ps.tile([P, 512], F32, tag="ps",
                                         name=f"ps1_s{s}_{half}_{t0}_{hh}")
                            pt.append(p_)
                        for d in range(KD):
                            w0off = (half * KD + d) * HHALF
                            for hh in range(KH // 2):
                                nc.tensor.matmul(
                                    pt[hh][:, :tw],
                                    w0sb[:, w0off + hh * P: w0off + (hh + 1) * P],
                                    xsb[:, d * T + t0: d * T + t0 + tw],
                                    start=(d == 0), stop=(d == KD - 1),
                                )
                        for hh in range(KH // 2):
                            h = half * (KH // 2) + hh
                            nc.scalar.activation(
                                hts[h][:, t0:t0 + tw], pt[hh][:, :tw],
                                AF.Gelu, bias=b0t[:, s * KH + h: s * KH + h + 1],
                            )

                # --- stage 2: yT[d, t] = W1^T @ hT, d on partitions, tokens
                # moving (no 128-token block padding). dc-outer so the
                # output can stream out in two contiguous half-D DMAs.
                yb = op.tile([P, KC * T], F32, tag="yb", name=f"yb_s{s}")
                for dc in range(KC):
                    for (t0, tw) in _t_chunks(T):
                        p2 = ps.tile([P, 512], F32, tag="ps",
                                     name=f"ps2_s{s}_{dc}_{t0}")
                        for h in range(KH):
                            nc.tensor.matmul(
                                p2[:, :tw],
                                w1sb[:, h * D + dc * P: h * D + (dc + 1) * P],
                                hts[h][:, t0:t0 + tw],
                                start=(h == 0), stop=(h == KH - 1),
                            )
                        nc.vector.tensor_copy(
                            yb[:, dc * T + t0: dc * T + t0 + tw], p2[:, :tw])
                    nc.scalar.dma_start(yt_out[:, dc * T:(dc + 1) * T],
                                        yb[:, dc * T:(dc + 1) * T])

    nc.compile()
    return nc


def _ensure_ntff_hook_module():
    """bass_utils unconditionally imports antenv.axon_hooks when tracing is
    requested; on images without it, provide a shim wired to the axon
    ctypes profiler when available (else a no-hook fallback)."""
    import importlib.util
    import sys
    import types

    if importlib.util.find_spec("antenv") is None:
        return
    try:
        import antenv.axon_hooks  # noqa: F401
        return
    except ImportError:
        pass
    mod = types.ModuleType("antenv.axon_hooks")
    mod._hook = None

    def set_axon_ntff_profile_hook(h):
        mod._hook = h

    def get_axon_ntff_profile_hook():
        return mod._hook

    mod.set_axon_ntff_profile_hook = set_axon_ntff_profile_hook
    mod.get_axon_ntff_profile_hook = get_axon_ntff_profile_hook
    try:
        from trn_agent_boot.trn_boot import _ntff_profile_via_ctypes
        mod._hook = _ntff_profile_via_ctypes("/opt/axon/libaxon_pjrt.so")
    except Exception:
        pass
    sys.modules["antenv.axon_hooks"] = mod
    import antenv
    antenv.axon_hooks = mod


def kernel(x, routing_tensor, W0, b0, W1, b1):
    global LAST_RESULTS
    import ml_dtypes
    from concourse.bass_utils import run_bass_kernel_spmd
    _ensure_ntff_hook_module()

    BF = ml_dtypes.bfloat16
    x = np.ascontiguousarray(x, dtype=np.float32)
    routing = np.asarray(routing_tensor, dtype=np.float32)
    W0 = np.asarray(W0, dtype=np.float32)
    b0 = np.asarray(b0, dtype=np.float32)
    W1 = np.asarray(W1, dtype=np.float32)
    b1 = np.asarray(b1, dtype=np.float32)

    # --- routing prep: per-expert token lists ---
    idx = [np.nonzero(routing[:, e])[0] for e in range(NE)]
    counts = np.array([len(i) for i in idx])
    order = np.argsort(-counts, kind="stable")  # experts sorted by load desc
    big, small = order[:8], order[8:][::-1]     # pair rank i with rank 15-i
    T0 = max(P, int(counts[big].max()) + 1 >> 1 << 1)
    T1 = max(P, int(counts[small].max()) + 1 >> 1 << 1)

    key = (T0, T1)
    if key not in _CACHE:
        _CACHE[key] = _build(T0, T1)
    nc = _CACHE[key]

    # --- build per-core inputs ---
    in_maps = []
    KD, KH, HHALF = D // P, H // P, H // 2
    for c in range(8):
        ea, eb = int(big[c]), int(small[c])
        m = {}
        for name, e, T in (("xt0", ea, T0), ("xt1", eb, T1)):
            g = np.zeros((T, D), np.float32)
            g[: len(idx[e])] = x[idx[e]]
            # [P, KD*T]: col d*T + t = x[t, d*128+p]
            m[name] = np.ascontiguousarray(
                g.T.reshape(KD, P, T).transpose(1, 0, 2).reshape(P, KD * T)
                .astype(BF))
        for name, e in (("w0a", ea), ("w0b", eb)):
            # [P, 2*KD*HHALF]: col (half*KD+d)*HHALF + j
            m[name] = np.ascontiguousarray(
                W0[e].reshape(KD, P, 2, HHALF).transpose(1, 2, 0, 3)
                .reshape(P, 2 * KD * HHALF).astype(BF))
        for name, e in (("w1a", ea), ("w1b", eb)):
            # [P, KH*D]: col h*D + j
            m[name] = np.ascontiguousarray(
                W1[e].reshape(KH, P, D).transpose(1, 0, 2)
                .reshape(P, KH * D).astype(BF))
        # device processes the small slot (eb) first: its bias in cols [:KH]
        b0m = np.zeros((P, 2 * KH), np.float32)
        b0m[:, :KH] = b0[eb].reshape(KH, P).T
        b0m[:, KH:] = b0[ea].reshape(KH, P).T
        m["b0s"] = b0m
        in_maps.append(m)

    res = run_bass_kernel_spmd(nc, in_maps, core_ids=list(range(8)),
                               trace=bool(os.environ.get("BASS_TRACE")))
    LAST_RESULTS = res

    # --- combine: out = routing @ b1 + score-scaled scatter-add of the
    # per-expert transposed outputs ---
    out = routing @ b1
    for c in range(8):
        ea, eb = int(big[c]), int(small[c])
        # yt [P, KC*T] -> y^T [D, T]: y^T[dc*128+p, t] = yt[p, dc*T+t]
        y0 = np.asarray(res.results[c]["yt0"]).reshape(P, D // P, T0) \
            .transpose(1, 0, 2).reshape(D, T0)
        y1 = np.asarray(res.results[c]["yt1"]).reshape(P, D // P, T1) \
            .transpose(1, 0, 2).reshape(D, T1)
        na, nb = len(idx[ea]), len(idx[eb])
        out[idx[ea]] += y0[:, :na].T * routing[idx[ea], ea][:, None]
        out[idx[eb]] += y1[:, :nb].T * routing[idx[eb], eb][:, None]
    return out.astype(np.float32)
